# revision 4
# baseline (speedup 1.0000x reference)
"""Causal self-attention (dense transformer block) on 8 Trainium2 NeuronCores, v2.

Sharding: batch (4) x head-halves (2) -> 8 cores; core c = batch c//2, heads
[8*(c%2), 8*(c%2)+8). Host sums the two partial output projections per batch.

v2 vs v1 (ACT-exp was the measured HW bottleneck, DMA traffic second):
  - all matmul operands bf16 (half DMA/SBUF traffic; fp32 PSUM accumulate)
  - exp batched 2 key-blocks x 2 heads wide ([128, 2, 2, 512] PSUM chunks,
    one 2048-col ACT instruction; measured ~1.1us vs 4 x 1.0us at 512)
  - diag-B chunks (fully-masked first half) trim scores/exp to [256:512];
    their prob tiles' dead half is DVE-memset zero so att@V runs full width
  - causal mask added via identity-matmul of a shifted master pattern
  - QKV bias rides the PSUM->SBUF copy as per-partition tensor_scalar_add
  - j-major single pass: QKV(tile j) -> attention(all pairs, qtile j) ->
    outproj rows; QKV/outproj chains interleave into the attention chunk
    stream as PE filler covering the exp latency (score pool is 1 deep)
  - outproj partials written bf16, DMA'd on the Pool SWDGE queue while
    inputs stream on the SP HWDGE queue
  - reps>1 timing loop uses staggered_reset: no all-engine barrier, next
    rep's weight/x DMAs prefetch under the current rep's tail
"""

import numpy as np

D = 1024
SEQ = 2048
NCORES = 8
PAIRS = 4
NJ = 4          # q-tiles of 512
JT = 512
NKB = 16        # key blocks of 128
MASKVAL = -1e9

_CACHE = {}


def _build_nc(reps: int = 1, phases: int = 3):
    import concourse.mybir as mybir
    import concourse.tile as tile
    from concourse import bacc

    f32 = mybir.dt.float32
    bf16 = mybir.dt.bfloat16

    nc = bacc.Bacc("TRN2", target_bir_lowering=False, debug=False)
    tensors = dict(
        xt_d=nc.dram_tensor("xt", [D, SEQ], bf16, kind="ExternalInput").ap(),
        wqkv_d=nc.dram_tensor("wqkv", [D, 1536], bf16, kind="ExternalInput").ap(),
        bqk_d=nc.dram_tensor("bqk", [128, 8], f32, kind="ExternalInput").ap(),
        bv_d=nc.dram_tensor("bv", [1, 8, 64], bf16, kind="ExternalInput").ap(),
        wo_d=nc.dram_tensor("wo", [512, D], bf16, kind="ExternalInput").ap(),
        ident_d=nc.dram_tensor("identc", [128, 128], bf16,
                               kind="ExternalInput").ap(),
        masters_d=nc.dram_tensor("masters", [128, 2, 2, 256], bf16,
                                 kind="ExternalInput").ap(),
        out_d=nc.dram_tensor("out", [SEQ, D], bf16, kind="ExternalOutput").ap(),
    )

    ET = mybir.EngineType

    with tile.TileContext(nc) as tc:
        with (
            tc.tile_pool(name="const", bufs=1) as constp,
            tc.tile_pool(name="qk", bufs=1) as qkp,
            tc.tile_pool(name="vt", bufs=1) as vp,
            tc.tile_pool(name="yt", bufs=1) as ytp,
            tc.tile_pool(name="w", bufs=2) as wp,
            tc.tile_pool(name="woql", bufs=2) as wop,
            tc.tile_pool(name="bias", bufs=2) as biasp,
            tc.tile_pool(name="xs", bufs=3) as xtp,
            tc.tile_pool(name="pr", bufs=4) as prp,
            tc.tile_pool(name="obp", bufs=2) as obp,
            tc.tile_pool(name="sums", bufs=2) as sup,
            tc.tile_pool(name="rbc", bufs=2) as rbp,
            tc.tile_pool(name="scp", bufs=2, space="PSUM") as scp,
            tc.tile_pool(name="ytps", bufs=2, space="PSUM") as ytpsp,
            tc.tile_pool(name="gp", bufs=2, space="PSUM") as gpp,
        ):
            # ---- constants + persistent tiles (loaded once, outside reps) ----
            identb = constp.tile([128, 128], bf16, name="identb")
            nc.sync.dma_start(out=identb, in_=tensors["ident_d"])
            masters = constp.tile([128, 2, 2, 256], bf16, name="masters")
            nc.sync.dma_start(out=masters, in_=tensors["masters_d"])
            onesb = constp.tile([1, 128], bf16, name="onesb")
            nc.vector.memset(onesb, 1.0)

            qts = [qkp.tile([128, SEQ], bf16, name=f"qt{p}") for p in range(PAIRS)]
            kts = [qkp.tile([128, SEQ], bf16, name=f"kt{p}") for p in range(PAIRS)]
            yts = [ytp.tile([128, SEQ], bf16, name=f"yt{p}") for p in range(PAIRS)]
            v_all = vp.tile([128, NKB, 8, 65], bf16, name="vall")
            # softmax-denominator ones column rides att@V as output row 64
            nc.vector.memset(v_all[:, :, :, 64:65], 1.0)

            pack = dict(
                f32=f32, bf16=bf16, identb=identb, masters=masters, onesb=onesb,
                qts=qts, kts=kts, yts=yts, v_all=v_all,
                wp=wp, wop=wop, biasp=biasp, xtp=xtp, prp=prp, obp=obp,
                sup=sup, rbp=rbp, scp=scp, ytpsp=ytpsp, gpp=gpp, **tensors)

            import os
            stagger = os.environ.get("KV2_STAGGER", "1") == "1"
            if reps > 1:
                with tc.For_i(0, reps, 1, staggered_reset=stagger,
                              hint_engines=(ET.PE, ET.Activation, ET.DVE,
                                            ET.Pool, ET.SP)):
                    _emit(nc, tc, mybir, pack, phases, staged=stagger)
            else:
                _emit(nc, tc, mybir, pack, phases, staged=False)
    nc.finalize()
    return nc


def _emit(nc, tc, mybir, L, phases, staged=False):
    f32, bf16 = L["f32"], L["bf16"]
    EXP = mybir.ActivationFunctionType.Exp
    identb, masters, onesb = L["identb"], L["masters"], L["onesb"]
    qts, kts, yts, v_all = L["qts"], L["kts"], L["yts"], L["v_all"]

    sub = {-1: 0, 1: 1, 4: 4, 5: 5, 6: 6, 2: 7, 3: 8}[phases]

    # ---- per-rep input DMAs; x tiles first (QKV needs them soonest) ----
    xts = {}

    def dma_xt(s):
        xt_s = L["xtp"].tile([128, 8, JT], bf16, tag="xt", name=f"xt{s}")
        nc.sync.dma_start(
            out=xt_s,
            in_=L["xt_d"].rearrange("(e p) s -> p e s", p=128)
            [:, :, s * JT:(s + 1) * JT])
        xts[s] = xt_s

    dma_xt(0)
    dma_xt(1)
    wqkv_sb = L["wp"].tile([128, 8, 1536], bf16, tag="wqkv")
    for e in range(8):
        nc.sync.dma_start(
            out=wqkv_sb[:, e, :],
            in_=L["wqkv_d"].rearrange("(e p) m -> p e m", p=128)[:, e, :])
    bqk_sb = L["biasp"].tile([128, 8], f32, tag="bqk")
    nc.sync.dma_start(out=bqk_sb, in_=L["bqk_d"])
    bv_sb = L["biasp"].tile([1, 8, 64], bf16, tag="bv")
    nc.sync.dma_start(out=bv_sb, in_=L["bv_d"])
    # wo is not needed until the first outproj (mid-rep): last on the queue
    wo_sb = L["wop"].tile([128, 4, D], bf16, tag="wo")
    for r in range(4):
        nc.sync.dma_start(
            out=wo_sb[:, r, :],
            in_=L["wo_d"].rearrange("(r p) n -> p r n", p=128)[:, r, :])

    def qkv_qk_pair(s0, p, qk):
        # Q or K chunk for seq tiles s0 and s0+1 sharing each weight load
        if sub < 1:
            return
        cols = 512 * qk + 128 * p
        psa = L["gpp"].tile([128, JT], f32, tag="g", name=f"qka{s0}{p}{qk}")
        psb = L["gpp"].tile([128, JT], f32, tag="g", name=f"qkb{s0}{p}{qk}")
        for e in range(8):
            w = wqkv_sb[:, e, cols:cols + 128]
            nc.tensor.matmul(psa, w, xts[s0][:, e, :],
                             start=(e == 0), stop=(e == 7))
            nc.tensor.matmul(psb, w, xts[s0 + 1][:, e, :],
                             start=(e == 0), stop=(e == 7))
        dst = (qts if qk == 0 else kts)[p]
        bias = bqk_sb[:, 2 * p + qk:2 * p + qk + 1]
        nc.vector.tensor_scalar_add(dst[:, s0 * JT:(s0 + 1) * JT], psa, bias)
        nc.vector.tensor_scalar_add(dst[:, (s0 + 1) * JT:(s0 + 2) * JT], psb,
                                    bias)

    def qkv_v_unit(s, b4):
        if sub < 1:
            return
        kb = 4 * s + b4
        psv = L["gpp"].tile([128, 8, 64], f32, tag="g", name=f"v{s}{b4}")
        nc.tensor.matmul(psv, onesb[0:1, 0:128], bv_sb, start=True, stop=False)
        for e in range(8):
            nc.tensor.matmul(psv, xts[s][:, e, 128 * b4:128 * b4 + 128],
                             wqkv_sb[:, e, 1024:1536], start=False, stop=(e == 7))
        nc.vector.tensor_copy(v_all[:, kb, :, 0:64], psv)

    def qkv_units(s0):
        # units covering seq tiles (s0, s0+1)
        units = []
        for p in range(PAIRS):
            for qk in range(2):
                units.append(lambda s0=s0, p=p, qk=qk: qkv_qk_pair(s0, p, qk))
            units.append(lambda s=s0, b4=p: qkv_v_unit(s, b4))
            units.append(lambda s=s0 + 1, b4=p: qkv_v_unit(s, b4))
        return units

    def op_unit(qt, ncol, ob):
        if sub < 8:
            return
        ps = L["gpp"].tile([128, 512], f32, tag="g", name=f"op{qt}{ncol}")
        for p in range(PAIRS):
            nc.tensor.matmul(ps, yts[p][:, 128 * qt:128 * qt + 128],
                             wo_sb[:, p, 512 * ncol:512 * ncol + 512],
                             start=(p == 0), stop=(p == PAIRS - 1))
        nc.vector.tensor_copy(ob[:, 512 * ncol:512 * ncol + 512], ps)
        if ncol == 1:
            nc.gpsimd.dma_start(out=L["out_d"][128 * qt:128 * qt + 128, :],
                                in_=ob)

    def op_units(j):
        units = []
        for qt in range(4 * j, 4 * j + 4):
            ob = L["obp"].tile([128, D], bf16, tag="ob", name=f"ob{qt}")
            for ncol in range(2):
                units.append(lambda qt=qt, ncol=ncol, ob=ob: op_unit(qt, ncol, ob))
        return units

    # ---- attention for (pair p, qtile j): stream of 2-keyblock chunks ----
    def attention(p, j, pop_filler):
        q0 = j * JT
        nkb = 4 * j + 4
        h2 = 2 * p
        yt_ps = [L["ytpsp"].tile([65, JT], f32, tag="ytps",
                                 name=f"ytps{p}{j}{h}") for h in range(2)]
        pending = []

        def emit_attnv(kb, pr, d0):
            if sub < 6:
                return
            # B chunks (d0=256) only touch cols [256:512); cols [0:256) close
            # their accumulation at kb=nkb-3 (the last full-width chunk)
            for h in range(2):
                nc.tensor.matmul(yt_ps[h][:, d0:JT], v_all[:, kb, h2 + h, :],
                                 pr[:, h, d0:JT],
                                 start=(kb == 0),
                                 stop=(kb >= nkb - 3), skip_group_check=True)

        for kb in range(nkb):
            d = 128 * kb - q0
            diag = d >= 0
            d0 = 256 if (diag and d >= 256) else 0
            i = (d // 128) % 2
            if sub >= 4:
                sc = L["scp"].tile([128, 2, JT], f32, tag="sc",
                                   name=f"sc{p}{j}{kb}")
                for h in range(2):
                    hs = 64 * h
                    nc.tensor.matmul(
                        sc[:, h, d0:JT],
                        kts[p][hs:hs + 64, 128 * kb:128 * kb + 128],
                        qts[p][hs:hs + 64, q0 + d0:q0 + JT],
                        start=True, stop=True)
                if diag:
                    # per-slice mask adds: matmul output must stay in one bank
                    for h in range(2):
                        nc.tensor.matmul(
                            sc[:, h, d0:d0 + 256], identb,
                            masters[:, i, h, :],
                            start=False, stop=True, skip_group_check=True)
            if sub >= 5:
                pr = L["prp"].tile([128, 2, JT], bf16, tag="pr",
                                   name=f"pr{p}{j}{kb}")
                nc.scalar.activation(out=pr[:, :, d0:JT],
                                     in_=sc[:, :, d0:JT],
                                     func=EXP, scale=0.125)
                pending.append((kb, pr, d0))
                if len(pending) > 2:
                    emit_attnv(*pending.pop(0))
            pop_filler()
        for args in pending:
            emit_attnv(*args)
        if sub < 7:
            return
        for h in range(2):
            den = L["sup"].tile([1, JT], f32, tag="den", name=f"den{p}{j}{h}")
            nc.vector.tensor_copy(den, yt_ps[h][64:65, :])
            rc = L["sup"].tile([1, JT], f32, tag="rc", name=f"rc{p}{j}{h}")
            nc.vector.reciprocal_approx_fast(rc, den)
            rb = L["rbp"].tile([64, JT], f32, tag="rb", name=f"rb{p}{j}{h}")
            nc.gpsimd.partition_broadcast(rb, rc)
            nc.vector.tensor_mul(
                yts[p][64 * h:64 * h + 64, q0:q0 + JT], yt_ps[h][0:64, :], rb)

    # ---------------- the single j-major pass ----------------
    for u in qkv_units(0):      # seq tiles 0,1
        u()
    for j in range(NJ):
        # fillers at j>=1 are outproj units gated on the previous tile's
        # normalize: hold them back a few chunks. QKV fillers (j=0) are free.
        LEAD = 2 if j == 0 else 8
        fillers = []
        if j == 0:
            dma_xt(2)
            dma_xt(3)
            fillers.extend(qkv_units(2))   # seq tiles 2,3
        if j >= 1:
            fillers.extend(op_units(j - 1))
        nchunks = PAIRS * (4 * j + 4)
        state = {"done": 0, "chunks": 0}

        def pop_filler():
            state["chunks"] += 1
            c = state["chunks"] - LEAD
            n = nchunks - LEAD
            want = len(fillers) * max(0, c) // n
            while state["done"] < want:
                fillers[state["done"]]()
                state["done"] += 1

        for p in range(PAIRS):
            attention(p, j, pop_filler)
        while state["done"] < len(fillers):
            fillers[state["done"]]()
            state["done"] += 1
        if staged and j < NJ - 1:
            tc.stage_boundary()
    for u in op_units(NJ - 1):
        u()


def get_nc(reps: int = 1, phases: int = 3):
    key = (reps, phases)
    if key not in _CACHE:
        _CACHE[key] = _build_nc(reps, phases)
    return _CACHE[key]


def shard_inputs(x, w_qkv, b_qkv, w_o):
    """Per-core input dicts for cores 0..7."""
    import ml_dtypes
    bf = ml_dtypes.bfloat16
    x = np.asarray(x, dtype=np.float32)
    w_qkv = np.asarray(w_qkv, dtype=np.float32)
    b_qkv = np.asarray(b_qkv, dtype=np.float32)
    w_o = np.asarray(w_o, dtype=np.float32)
    xts = [np.ascontiguousarray(x[b].T).astype(bf) for b in range(x.shape[0])]
    ident = np.eye(128, dtype=np.float32).astype(bf)
    kk = np.arange(128)[:, None, None, None]
    ii = np.arange(2)[None, :, None, None]
    mm = np.arange(256)[None, None, None, :]
    masters = np.where(mm < 128 * ii + kk, np.float32(MASKVAL),
                       0.0).astype(bf)
    masters = np.broadcast_to(masters, (128, 2, 2, 256)).copy()
    in_maps = []
    for c in range(NCORES):
        b, g = divmod(c, 2)
        cols = slice(512 * g, 512 * g + 512)
        wq, wk, wv = (w_qkv[:, 1024 * i:1024 * (i + 1)][:, cols]
                      for i in range(3))
        bq, bk, bv = (b_qkv[1024 * i:1024 * (i + 1)][cols] for i in range(3))
        bqk = np.zeros((128, 8), dtype=np.float32)
        for p in range(PAIRS):
            bqk[:, 2 * p] = bq[128 * p:128 * p + 128]
            bqk[:, 2 * p + 1] = bk[128 * p:128 * p + 128]
        in_maps.append({
            "xt": xts[b],
            "wqkv": np.ascontiguousarray(
                np.concatenate([wq, wk, wv], axis=1)).astype(bf),
            "bqk": bqk,
            "bv": bv.reshape(1, 8, 64).astype(bf),
            "wo": np.ascontiguousarray(w_o[512 * g:512 * g + 512, :]).astype(bf),
            "identc": ident,
            "masters": masters,
        })
    return in_maps


def kernel(x, w_qkv, b_qkv, w_o, b_o):
    from concourse.bass_utils import run_bass_kernel_spmd

    nc = get_nc()
    in_maps = shard_inputs(x, w_qkv, b_qkv, w_o)
    res = run_bass_kernel_spmd(nc, in_maps, core_ids=list(range(NCORES)))
    parts = [r["out"] for r in res.results]
    b_o = np.asarray(b_o, dtype=np.float64)
    out = np.empty((4, SEQ, D), dtype=np.float32)
    for b in range(4):
        out[b] = (parts[2 * b].astype(np.float64)
                  + parts[2 * b + 1].astype(np.float64) + b_o).astype(np.float32)
    return out
